# revision 38
# baseline (speedup 1.0000x reference)
# Trainium2 Bass kernel for a BitLinear transformer block (attention + SwiGLU FFN).
#
# Sharding across 8 NeuronCores (hybrid):
#   - Attention: head-parallel. Core c computes q/k/v + causal attention for
#     global heads {2c, 2c+1}, both batches, over ALL tokens.
#   - out_proj / rmsnorm2 / FFN: token-parallel. Core c handles 256 tokens of
#     batch 0 ([256c, 256c+256)) and the same range of batch 1, with the FULL
#     weight matrices.
#   - Collectives: two half AllGathers of the rmsnorm'd activations (computed
#     sequence-parallel, shipped feature-major in fp8), and two AllToAlls (one
#     per batch, fp8) that re-shard attention outputs from head-parallel to
#     token-parallel. The batch splits let batch-0's collective and out_proj
#     overlap batch-1's attention.
#
# Weights are pre-quantized on the host to ternary {-1,0,1} in matmul-ready
# transposed layouts; ternary values are exact in fp8-e4m3, so the qkv /
# out_proj / gate / up / down weights ship as fp8 (half the HBM traffic of
# bf16 at identical matmul speed). Activations on the quantized-matmul paths
# (xhat, attention out, x2n) are fp8 as well; q/k/v, softmax, and the SwiGLU
# product stay bf16; all matmuls accumulate in fp32 PSUM and the residual
# stream stays fp32 end-to-end. Softmax runs in fp32 without max-subtraction
# (logits are O(1) at this problem's scale). Partition-broadcasts (rsqrt of
# rmsnorm, softmax row-normalizers) are rank-1 ones-matmuls on the tensor
# engine instead of DRAM round-trips.

import numpy as np

B, T, D, H, Dh, F = 2, 2048, 1024, 16, 64, 4096
BT = B * T
NC_ = 8
TLOC = BT // NC_          # 512 tokens per core (256 of each batch)
SUB = 256                 # tokens per (core, batch)
EPS = 1e-6

N_AG = 128 * 8 * SUB      # one AllGather shard: [128 p, 8 dk, 256 t] fp8


def _patch_tile_tail():
    # This container's walrus rejects the InstISA sem_clear/dma_reset that
    # TileContext emits at kernel tail ("ISA wrong length"). The clears only
    # matter for re-executing a loaded NEFF; skip emitting them and keep the
    # bookkeeping.
    import concourse.bass as bass
    if getattr(bass.Bass, "_acfs_patched", False):
        return
    def _cfs(self, sems):
        if not sems:
            return
        sem_nums = [s.num if hasattr(s, "num") else s for s in sems]
        self._state.prepend_free_semaphores(sem_nums)
        for poison_set in self._tile_sem_poison_stack:
            poison_set.update(sem_nums)
    bass.Bass.clear_and_free_semaphores = _cfs
    bass.Bass._acfs_patched = True


def _legalize_multiwaits(nc):
    # This container's walrus encodes at most ONE semaphore wait per
    # instruction. Tile attaches several. Split: hoist all but the last wait
    # into standalone single-wait EventSemaphore instructions on the same
    # engine, immediately before the original instruction (same block, so
    # per-engine program order is preserved).
    import concourse.mybir as mybir
    wid = 0
    for bb in nc.main_func.blocks:
        il = bb.instructions
        new_list = []
        for inst in il:
            si = getattr(inst, "sync_info", None)
            if si is not None and si.on_wait is not None and len(si.on_wait) > 1:
                waits = list(si.on_wait)
                for w in waits[:-1]:
                    ev = mybir.InstEventSemaphore(name=f"WSPLIT-{wid}", ins=[], outs=[])
                    wid += 1
                    ev.engine = inst.engine
                    ev.sync_info = mybir.SyncInfo(on_wait=[w], on_update=[])
                    new_list.append(ev)
                inst.sync_info = mybir.SyncInfo(on_wait=[waits[-1]],
                                                on_update=list(si.on_update))
            new_list.append(inst)
        il[:] = new_list


def _build(scales):
    import concourse.bass as bass
    import concourse.mybir as mybir
    import concourse.tile as tile
    from concourse.masks import make_identity

    _patch_tile_tail()

    f32 = mybir.dt.float32
    bf16 = mybir.dt.bfloat16
    f8 = mybir.dt.float8e4
    AF = mybir.ActivationFunctionType
    ALU = mybir.AluOpType
    SQ, SO, SG, SU, SD = (float(scales[k]) for k in ("qkv", "out", "gate", "up", "down"))
    DR = mybir.MatmulPerfMode.DoubleRow

    nc = bass.Bass(num_devices=NC_)
    RG = [list(range(NC_))]

    # ---- I/O (fp8 weights are host-side ternary-quantized, pre-transposed) ----
    x_slice = nc.dram_tensor("x_slice", [TLOC, D], f32, kind="ExternalInput")
    wqkvT = nc.dram_tensor("wqkvT", [128, 3 * 8 * 128], f8, kind="ExternalInput")
    woT = nc.dram_tensor("woT", [128, 8 * 1024], f8, kind="ExternalInput")
    wgT = nc.dram_tensor("wgT", [D, F], f8, kind="ExternalInput")
    wuT = nc.dram_tensor("wuT", [D, F], f8, kind="ExternalInput")
    wdT = nc.dram_tensor("wdT", [F, D], f8, kind="ExternalInput")
    out_d = nc.dram_tensor("out", [TLOC, D], f32, kind="ExternalOutput")

    def ap(t, off, dims):
        return bass.AP(tensor=t.tensor, offset=t.offset + off, ap=dims)

    with tile.TileContext(nc) as tc:
        import contextlib
        ctx = contextlib.ExitStack()
        with ctx:
            dram = ctx.enter_context(tc.tile_pool(name="dram", bufs=1, space="DRAM"))
            sing = ctx.enter_context(tc.tile_pool(name="sing", bufs=1))
            psS_p = ctx.enter_context(tc.tile_pool(name="psS", bufs=2, space="PSUM"))
            psO_p = ctx.enter_context(tc.tile_pool(name="psO", bufs=2, space="PSUM"))
            psA_p = ctx.enter_context(tc.tile_pool(name="psA", bufs=2, space="PSUM"))
            pool = ctx.enter_context(tc.tile_pool(name="pool", bufs=2))

            # ---- DRAM internals ----
            ag_in = [dram.tile([N_AG], f8, name=f"ag{i}_in") for i in range(2)]
            ag_out = [dram.tile([NC_ * N_AG], f8, name=f"ag{i}_out",
                                addr_space="Shared") for i in range(2)]
            a2a_in = [dram.tile([NC_, 128, SUB], f8, name=f"a2a{i}_in")
                      for i in range(2)]
            a2a_out = [dram.tile([NC_, 128, SUB], f8, name=f"a2a{i}_out")
                       for i in range(2)]

            # ---- persistent SBUF ----
            id_bf = sing.tile([128, 128], bf16, name="id_bf")
            make_identity(nc, id_bf)
            id_f32 = sing.tile([128, 128], f32, name="id_f32")
            make_identity(nc, id_f32)
            ones_col = sing.tile([128, 1], bf16, name="ones_col")
            nc.vector.memset(ones_col, 1.0)
            # ones at base partition 64 (bf16) for the softmax-denominator
            # broadcast matmul, whose rhs lives on partition 64
            ones65 = sing.tile([65, 128], bf16, name="ones65")
            nc.vector.memset(ones65, 1.0)
            # causal keep-mask M[p, u] = 1.0 iff p <= u - 384   (bf16, [128, 1024])
            mask_big = sing.tile([128, 1024], bf16, name="mask_big")
            nc.gpsimd.memset(mask_big, 1.0)
            nc.gpsimd.affine_select(
                out=mask_big, in_=mask_big, compare_op=ALU.is_ge, fill=0.0,
                base=-384, channel_multiplier=-1, pattern=[[1, 1024]],
            )
            eps_t = sing.tile([128, 1], f32, name="eps_t")
            nc.vector.memset(eps_t, EPS)

            wqkv_sb = sing.tile([128, 3, 8, 128], f8, name="wqkv_sb")
            wo_sb = sing.tile([128, 8, 1024], f8, name="wo_sb")
            qk_sb = sing.tile([128, 2, BT], bf16, name="qk_sb")   # q,k feature-major
            v_tm = sing.tile([128, 4, 16, 65], bf16, name="v_tm")  # per (b,hl): token-major v + ones col
            nc.vector.memset(v_tm[:, :, :, 64:65], 1.0)
            x_fm = sing.tile([128, 8, TLOC], f32, name="x_fm")     # residual stream, feature-major
            x2n = sing.tile([128, 8, TLOC], f8, name="x2n")
            a2a_sb = sing.tile([128, 8, TLOC], f8, name="a2a_sb")
            rstd1 = sing.tile([1, TLOC], f32, name="rstd1")
            rstd2 = sing.tile([1, TLOC], f32, name="rstd2")

            # ============ Stage A: x slice -> feature-major, rmsnorm1, AG ============
            # processed per batch-half so the first AllGather fires ASAP
            xh_fm = pool.tile([128, 2, 8, SUB], f8, name="xh_fm", tag="xhout", bufs=1)
            for bh in range(2):
                lo = bh * SUB
                for tt in range(2):
                    xs = pool.tile([128, 1024], f32, name="xs", tag="raw4", bufs=2)
                    nc.sync.dma_start(
                        out=xs, in_=x_slice[lo + tt * 128: lo + (tt + 1) * 128, :])
                    for dkq in range(2):
                        ps = psA_p.tile([128, 512], f32, name="psA", tag="psA")
                        for kk in range(4):
                            dk = dkq * 4 + kk
                            nc.tensor.transpose(ps[:, kk * 128:(kk + 1) * 128],
                                                xs[:, dk * 128:(dk + 1) * 128], id_f32)
                        nc.vector.tensor_copy(
                            x_fm[:, dkq * 4:(dkq + 1) * 4, lo + tt * 128: lo + (tt + 1) * 128],
                            ps.rearrange("p (a b) -> p a b", b=128))
                psn = psA_p.tile([1, SUB], f32, name="psA", tag="psA")
                for m in range(8):
                    sq = pool.tile([128, SUB], bf16, name="sq", tag="sqb", bufs=3)
                    nc.vector.tensor_mul(sq, x_fm[:, m, lo:lo + SUB],
                                         x_fm[:, m, lo:lo + SUB])
                    nc.tensor.matmul(psn, ones_col, sq, start=(m == 0), stop=(m == 7))
                # sqrt -> bf16, broadcast via ones-matmul, reciprocal of the
                # broadcast (vector-parallel over 128 lanes), then scale
                r1 = rstd1[:, lo:lo + SUB]
                nc.scalar.activation(r1, psn, AF.Sqrt, scale=1.0 / D,
                                     bias=eps_t[0:1, :])
                r1b = pool.tile([1, SUB], bf16, name="r1b", tag="r1b", bufs=2)
                nc.vector.tensor_copy(r1b, r1)
                psb = psA_p.tile([128, SUB], f32, name="psA", tag="psA")
                nc.tensor.matmul(psb, ones65[0:1, :], r1b, start=True, stop=True)
                rb = pool.tile([128, SUB], f32, name="rb", tag="rb", bufs=2)
                nc.vector.reciprocal(rb, psb)
                for m in range(8):
                    nc.vector.tensor_mul(xh_fm[:, bh, m, :],
                                         x_fm[:, m, lo:lo + SUB], rb)
                nc.sync.dma_start(
                    out=ap(ag_in[bh], 0, [[8 * SUB, 128], [1, 8 * SUB]]),
                    in_=xh_fm[:, bh, :, :].rearrange("p a b -> p (a b)"))
                nc.gpsimd.collective_compute(
                    "AllGather", ALU.bypass, replica_groups=RG,
                    ins=[ag_in[bh][:].opt()], outs=[ag_out[bh][:].opt()])

            # weight loads (pure DMA; no on-device quantization needed)
            nc.sync.dma_start(out=wqkv_sb.rearrange("p a b c -> p (a b c)"),
                              in_=wqkvT[:, :])
            nc.sync.dma_start(out=wo_sb.rearrange("p a b -> p (a b)"), in_=woT[:, :])

            # ============ qkv (per AG chunk) + attention, interleaved ============
            def qkv_chunk(bh, r):
                # chunk r of AG bh: xhat feature-major for batch-bh tokens
                # [256r, 256r+256); produce q/k (feature-major) + v (token-major)
                xh_sb = pool.tile([128, 8, SUB], f8, name="xh_sb", tag="xh8",
                                  bufs=3)
                nc.sync.dma_start(
                    out=xh_sb.rearrange("p a b -> p (a b)"),
                    in_=ap(ag_out[bh], r * N_AG, [[8 * SUB, 128], [1, 8 * SUB]]))
                for m in range(3):
                    ps = psA_p.tile([128, SUB], f32, name="psA", tag="psA")
                    for a in range(4):
                        nc.tensor.matmul(
                            ps, wqkv_sb[:, m, 2 * a:2 * a + 2, :],
                            xh_sb[:, 2 * a:2 * a + 2, :],
                            start=(a == 0), stop=(a == 3), perf_mode=DR)
                    if m < 2:
                        nc.vector.tensor_copy(
                            qk_sb[:, m, bh * 2048 + r * SUB: bh * 2048 + (r + 1) * SUB],
                            ps)
                    else:
                        v_ch = pool.tile([128, SUB], bf16, name="v_ch", tag="vch", bufs=2)
                        nc.vector.tensor_copy(v_ch, ps)
                        psv = psA_p.tile([128, SUB], bf16, name="psA", tag="psA")
                        for jj in range(2):
                            nc.tensor.transpose(psv[:, jj * 128:(jj + 1) * 128],
                                                v_ch[:, jj * 128:(jj + 1) * 128], id_bf)
                        psv_j = psv.rearrange("p (j x) -> p j x", x=128)
                        for hl in range(2):
                            # psv cols: [jj 2 x (hl0 64 | hl1 64)] -> v_tm j 2r, 2r+1
                            nc.vector.tensor_copy(
                                v_tm[:, bh * 2 + hl, 2 * r:2 * r + 2, 0:64],
                                psv_j[:, :, hl * 64:hl * 64 + 64])

            def attention_n(b, n):
                q0 = qk_sb[0:64, 0, b * 2048:(b + 1) * 2048]
                q1 = qk_sb[64:128, 0, b * 2048:(b + 1) * 2048]
                k0 = qk_sb[0:64, 1, b * 2048:(b + 1) * 2048]
                k1 = qk_sb[64:128, 1, b * 2048:(b + 1) * 2048]
                pso = [psO_p.tile([65, 512], f32, name="psO", tag="psO")
                       for _ in range(2)]
                jmax = 4 * n + 3
                av_q = []
                for j in range(jmax + 1):
                    pss = psS_p.tile([128, 1024], f32, name="psS", tag="psS")
                    nc.tensor.matmul(pss[:, 0:512], k0[:, j * 128:(j + 1) * 128],
                                     q0[:, n * 512:(n + 1) * 512], start=True, stop=True)
                    nc.tensor.matmul(pss[:, 512:1024], k1[:, j * 128:(j + 1) * 128],
                                     q1[:, n * 512:(n + 1) * 512], start=True, stop=True)
                    # emit the previous iteration's AV here so it sits behind
                    # this QK in the tensor queue; its exp/mask ran meanwhile
                    if len(av_q) >= 1:
                        av_q.pop(0)()
                    pt = pool.tile([128, 1024], bf16, name="pt", tag="pt", bufs=4)
                    nc.scalar.activation(pt, pss, AF.Exp, scale=SQ * SQ * (Dh ** -0.5))
                    if j >= 4 * n:
                        off = 512 * n - 128 * j + 384
                        nc.gpsimd.tensor_mul(pt[:, 0:512], pt[:, 0:512],
                                             mask_big[:, off:off + 512])
                        nc.gpsimd.tensor_mul(pt[:, 512:1024], pt[:, 512:1024],
                                             mask_big[:, off:off + 512])

                    def mk_av(j, pt):
                        def go():
                            for hl in range(2):
                                nc.tensor.matmul(
                                    pso[hl], v_tm[:, b * 2 + hl, j, :],
                                    pt[:, hl * 512:(hl + 1) * 512],
                                    start=(j == 0), stop=(j == jmax))
                        return go
                    av_q.append(mk_av(j, pt))
                for go in av_q:
                    go()
                # evict; cast the rowsum row to bf16 on partition 64. The
                # broadcast matmul then depends only on the cheap cast; the
                # expensive per-element division happens on vector via a
                # fused divide, emitted one n-block later (finisher).
                tails = []
                for hl in range(2):
                    o_sb = pool.tile([65, 512], f32, name="o_sb", tag="osb", bufs=5)
                    nc.vector.tensor_copy(o_sb, pso[hl])
                    # 1/rowsum = exp(-ln(x)) on scalar, straight to bf16
                    lnr = pool.tile([65, 512], f32, name="lnr", tag="lnr", bufs=3)
                    nc.scalar.activation(lnr[64:65, :], o_sb[64:65, :], AF.Ln)
                    rsbf = pool.tile([65, 512], bf16, name="rsbf", tag="rsbf", bufs=5)
                    nc.scalar.activation(rsbf[64:65, :], lnr[64:65, :], AF.Exp,
                                         scale=-1.0)
                    tails.append((o_sb, rsbf))

                def finish():
                    for hl, (o_sb, rsbf) in enumerate(tails):
                        psb2 = psA_p.tile([64, 512], f32, name="psA", tag="psA")
                        nc.tensor.matmul(psb2, ones65[64:65, 0:64], rsbf[64:65, :],
                                         start=True, stop=True)
                        o_n = pool.tile([64, 512], f8, name="o_n", tag="on", bufs=3)
                        nc.vector.scalar_tensor_tensor(
                            out=o_n, in0=o_sb[0:64, :], scalar=SQ,
                            op0=ALU.mult, op1=ALU.mult, in1=psb2)
                        for half in range(2):
                            nc.sync.dma_start(
                                out=ap(a2a_in[b],
                                       (2 * n + half) * 128 * SUB + hl * 64 * SUB,
                                       [[SUB, 64], [1, SUB]]),
                                in_=o_n[:, half * SUB:(half + 1) * SUB])
                return finish

            def outproj_norm2(b):
                lo, hi = b * SUB, (b + 1) * SUB
                for m in range(8):
                    ps = psA_p.tile([128, SUB], f32, name="psA", tag="psA")
                    for a in range(4):
                        nc.tensor.matmul(
                            ps, wo_sb[:, 2 * a:2 * a + 2, m * 128:(m + 1) * 128],
                            a2a_sb[:, 2 * a:2 * a + 2, lo:hi],
                            start=(a == 0), stop=(a == 3), perf_mode=DR)
                    nc.vector.scalar_tensor_tensor(
                        out=x_fm[:, m, lo:hi], in0=ps, scalar=SO, op0=ALU.mult,
                        op1=ALU.add, in1=x_fm[:, m, lo:hi])
                psn2 = psA_p.tile([1, SUB], f32, name="psA", tag="psA")
                for m in range(8):
                    sq2 = pool.tile([128, SUB], bf16, name="sq2", tag="sq2", bufs=3)
                    nc.vector.tensor_mul(sq2, x_fm[:, m, lo:hi], x_fm[:, m, lo:hi])
                    nc.tensor.matmul(psn2, ones_col, sq2, start=(m == 0), stop=(m == 7))
                r2 = rstd2[:, lo:hi]
                nc.scalar.activation(r2, psn2, AF.Sqrt, scale=1.0 / D,
                                     bias=eps_t[0:1, :])
                r2b = pool.tile([1, SUB], bf16, name="r2b", tag="r1b", bufs=2)
                nc.vector.tensor_copy(r2b, r2)
                psb3 = psA_p.tile([128, SUB], f32, name="psA", tag="psA")
                nc.tensor.matmul(psb3, ones65[0:1, :], r2b, start=True, stop=True)
                rb3 = pool.tile([128, SUB], f32, name="rb3", tag="rb", bufs=2)
                nc.vector.reciprocal(rb3, psb3)
                for m in range(8):
                    nc.vector.tensor_mul(x2n[:, m, lo:hi], x_fm[:, m, lo:hi], rb3)

            def fire_a2a(b):
                nc.gpsimd.collective_compute(
                    "AllToAll", ALU.bypass, replica_groups=RG,
                    ins=[a2a_in[b][:].opt()], outs=[a2a_out[b][:].opt()])
                nc.sync.dma_start(out=a2a_sb[:, :, b * SUB:(b + 1) * SUB],
                                  in_=a2a_out[b].rearrange("r p t -> p r t"))

            def load_gu(half, gl):
                gf0 = half * 16 + gl * 8
                wg_t = pool.tile([128, 8, 1024], f8, name="wg_t", tag="w8", bufs=4)
                for dk in range(8):
                    nc.sync.dma_start(
                        out=wg_t[:, dk, :],
                        in_=wgT[dk * 128:(dk + 1) * 128, gf0 * 128:gf0 * 128 + 1024])
                wu_t = pool.tile([128, 8, 1024], f8, name="wu_t", tag="w8", bufs=4)
                for dk in range(8):
                    nc.sync.dma_start(
                        out=wu_t[:, dk, :],
                        in_=wuT[dk * 128:(dk + 1) * 128, gf0 * 128:gf0 * 128 + 1024])
                return wg_t, wu_t

            def gu_block(wg_t, wu_t, a_sb, gl, c0, c1):
                # F-tiles gl*8..gl*8+8 of the half, token cols [c0, c1)
                for q2 in range(2):
                    sgs = []
                    for mm in range(4):
                        fi = q2 * 4 + mm
                        psg = psA_p.tile([128, c1 - c0], f32, name="psA", tag="psA")
                        for a in range(4):
                            nc.tensor.matmul(
                                psg, wg_t[:, 2 * a:2 * a + 2, fi * 128:(fi + 1) * 128],
                                x2n[:, 2 * a:2 * a + 2, c0:c1],
                                start=(a == 0), stop=(a == 3), perf_mode=DR)
                        sg = pool.tile([128, c1 - c0], bf16, name="sg", tag="sg", bufs=5)
                        nc.scalar.activation(sg, psg, AF.Silu, scale=SG)
                        sgs.append(sg)
                    for mm in range(4):
                        fi = q2 * 4 + mm
                        psu = psA_p.tile([128, c1 - c0], f32, name="psA", tag="psA")
                        for a in range(4):
                            nc.tensor.matmul(
                                psu, wu_t[:, 2 * a:2 * a + 2, fi * 128:(fi + 1) * 128],
                                x2n[:, 2 * a:2 * a + 2, c0:c1],
                                start=(a == 0), stop=(a == 3), perf_mode=DR)
                        nc.vector.scalar_tensor_tensor(
                            out=a_sb[:, gl * 8 + fi, c0:c1], in0=psu, scalar=SU,
                            op0=ALU.mult, op1=ALU.mult, in1=sgs[mm])

            # --- batch 0: qkv + attention (normalization tails pipelined) ---
            pend = None
            for n in range(4):
                qkv_chunk(0, 2 * n)
                qkv_chunk(0, 2 * n + 1)
                fin = attention_n(0, n)
                if pend:
                    pend()
                pend = fin
            pend()
            fire_a2a(0)
            # --- batch 1 attention; batch-0 out_proj/norm2 overlaps its middle ---
            pend = None
            for n in range(4):
                qkv_chunk(1, 2 * n)
                qkv_chunk(1, 2 * n + 1)
                fin = attention_n(1, n)
                if pend:
                    pend()
                pend = fin
                if n == 1:
                    outproj_norm2(0)
            # prefetch first FFN gate/up weights (DMA only), finish the last
            # attention tail, fire the second AllToAll, then fill its window
            # with the first gate/up block on batch-0 tokens
            a_sb0 = pool.tile([128, 16, TLOC], bf16, name="a_sb", tag="asb", bufs=2)
            wg0, wu0 = load_gu(0, 0)
            pend()
            fire_a2a(1)
            gu_block(wg0, wu0, a_sb0, 0, 0, SUB)
            outproj_norm2(1)

            # ============ FFN (two F-halves; down accumulated into x_fm) ============
            for half in range(2):
                a_sb = a_sb0 if half == 0 else pool.tile(
                    [128, 16, TLOC], bf16, name="a_sb", tag="asb", bufs=2)
                for gl in range(2):
                    if half == 0 and gl == 0:
                        gu_block(wg0, wu0, a_sb, 0, SUB, TLOC)
                        continue
                    wg_t, wu_t = load_gu(half, gl)
                    gu_block(wg_t, wu_t, a_sb, gl, 0, TLOC)
                wds = []
                for dh in range(2):
                    wd_t = pool.tile([128, 8, 1024], f8, name="wd_t", tag="w8", bufs=4)
                    for f8_ in range(8):
                        fk = half * 16 + dh * 8 + f8_
                        nc.sync.dma_start(out=wd_t[:, f8_, :],
                                          in_=wdT[fk * 128:(fk + 1) * 128, :])
                    wds.append(wd_t)
                for mo in range(8):
                    psd = psA_p.tile([128, 512], f32, name="psA", tag="psA")
                    for ff in range(16):
                        nc.tensor.matmul(
                            psd, wds[ff // 8][:, ff % 8, mo * 128:(mo + 1) * 128],
                            a_sb[:, ff, :], start=(ff == 0), stop=(ff == 15))
                    nc.vector.scalar_tensor_tensor(
                        out=x_fm[:, mo, :], in0=psd, scalar=SD, op0=ALU.mult,
                        op1=ALU.add, in1=x_fm[:, mo, :])
                    # once a 4-tile do-group is final (second F-half), emit its
                    # output transposes + stores immediately
                    if half == 1 and mo % 4 == 3:
                        doh = mo // 4
                        for tt in range(4):
                            ps = psA_p.tile([128, 512], f32, name="psA", tag="psA")
                            for mm in range(4):
                                nc.tensor.transpose(
                                    ps[:, mm * 128:(mm + 1) * 128],
                                    x_fm[:, doh * 4 + mm, tt * 128:(tt + 1) * 128],
                                    id_f32)
                            ob = pool.tile([128, 512], f32, name="ob", tag="f2k",
                                           bufs=3)
                            nc.vector.tensor_copy(ob, ps)
                            nc.sync.dma_start(
                                out=out_d[tt * 128:(tt + 1) * 128,
                                          doh * 512:(doh + 1) * 512],
                                in_=ob)
    _legalize_multiwaits(nc)
    return nc


def _quant(w, s):
    # host-side ternary quantization (exact in fp8-e4m3); matches
    # jnp.clip(jnp.round(W/s), -1, 1) including round-half-to-even
    return np.clip(np.round(w / s), -1.0, 1.0).astype(np.float32)


def _prepare(inputs):
    import ml_dtypes
    f8 = ml_dtypes.float8_e4m3
    x = np.asarray(inputs["x"], np.float32).reshape(BT, D)
    qkv_w = np.asarray(inputs["qkv_w"], np.float32)
    out_w = np.asarray(inputs["out_w"], np.float32)
    gate_w = np.asarray(inputs["gate_w"], np.float32)
    up_w = np.asarray(inputs["up_w"], np.float32)
    down_w = np.asarray(inputs["down_w"], np.float32)
    ln1 = np.asarray(inputs["ln1_w"], np.float32)
    ln2 = np.asarray(inputs["ln2_w"], np.float32)

    scales = {
        "qkv": max(np.mean(np.abs(qkv_w), dtype=np.float32), np.float32(1e-5)),
        "out": max(np.mean(np.abs(out_w), dtype=np.float32), np.float32(1e-5)),
        "gate": max(np.mean(np.abs(gate_w), dtype=np.float32), np.float32(1e-5)),
        "up": max(np.mean(np.abs(up_w), dtype=np.float32), np.float32(1e-5)),
        "down": max(np.mean(np.abs(down_w), dtype=np.float32), np.float32(1e-5)),
    }
    qkv_q = _quant(qkv_w, scales["qkv"]) * ln1[None, :]     # fold g1 into cols (d)
    out_q = _quant(out_w, scales["out"])
    gate_q = _quant(gate_w, scales["gate"]) * ln2[None, :]  # fold g2 into cols (d)
    up_q = _quant(up_w, scales["up"]) * ln2[None, :]
    down_q = _quant(down_w, scales["down"])

    # shared (identical on all cores), matmul-ready transposed layouts
    # woT[p, r, do] = out_q[do, r*128+p]
    woT = np.ascontiguousarray(
        out_q.T.reshape(8, 128, 1024).transpose(1, 0, 2).reshape(128, 8192)
    ).astype(f8)
    wgT = np.ascontiguousarray(gate_q.T).astype(f8)         # [1024 d, 4096 F]
    wuT = np.ascontiguousarray(up_q.T).astype(f8)
    wdT = np.ascontiguousarray(down_q.T).astype(f8)         # [4096 F, 1024 do]

    in_maps = []
    for c in range(NC_):
        # wqkvT[p, m, dk, f] = qkv_q[m*1024 + c*128 + f, dk*128 + p]
        wq = np.stack([
            qkv_q[m * 1024 + c * 128: m * 1024 + (c + 1) * 128, :]
            .T.reshape(8, 128, 128).transpose(1, 0, 2)
            for m in range(3)], axis=1)                      # [128, 3, 8, 128]
        in_maps.append({
            "x_slice": np.ascontiguousarray(np.concatenate(
                [x[SUB * c: SUB * (c + 1)],
                 x[T + SUB * c: T + SUB * (c + 1)]], axis=0)),
            "wqkvT": np.ascontiguousarray(wq.reshape(128, 3072)).astype(f8),
            "woT": woT,
            "wgT": wgT,
            "wuT": wuT,
            "wdT": wdT,
        })
    return scales, in_maps


def run(inputs, trace=False):
    from concourse.bass_utils import run_bass_kernel_spmd
    scales, in_maps = _prepare(inputs)
    nc = _build(scales)
    res = run_bass_kernel_spmd(nc, in_maps, list(range(NC_)), trace=trace)
    out = np.empty((BT, D), np.float32)
    for c in range(NC_):
        o = np.asarray(res.results[c]["out"])
        out[SUB * c: SUB * (c + 1)] = o[0:SUB]
        out[T + SUB * c: T + SUB * (c + 1)] = o[SUB:2 * SUB]
    return out.reshape(B, T, D), res


def kernel(**inputs):
    out, _ = run(inputs, trace=False)
    return out


# revision 39
# speedup vs baseline: 1.0646x; 1.0646x over previous
# Trainium2 Bass kernel for a BitLinear transformer block (attention + SwiGLU FFN).
#
# Sharding across 8 NeuronCores (hybrid):
#   - Attention: head-parallel. Core c computes q/k/v + causal attention for
#     global heads {2c, 2c+1}, both batches, over ALL tokens.
#   - out_proj / rmsnorm2 / FFN: token-parallel. Core c handles 256 tokens of
#     batch 0 ([256c, 256c+256)) and the same range of batch 1, with the FULL
#     weight matrices.
#   - Collectives: two half AllGathers of the rmsnorm'd activations (computed
#     sequence-parallel, shipped feature-major in fp8), and two AllToAlls (one
#     per batch, fp8) that re-shard attention outputs from head-parallel to
#     token-parallel. The batch splits let batch-0's collective and out_proj
#     overlap batch-1's attention.
#
# Weights are pre-quantized on the host to ternary {-1,0,1} in matmul-ready
# transposed layouts; ternary values are exact in fp8-e4m3, so the qkv /
# out_proj / gate / up / down weights ship as fp8 (half the HBM traffic of
# bf16 at identical matmul speed). Activations on the quantized-matmul paths
# (xhat, attention out, x2n) are fp8 as well; q/k/v, softmax, and the SwiGLU
# product stay bf16; all matmuls accumulate in fp32 PSUM and the residual
# stream stays fp32 end-to-end. Softmax runs in fp32 without max-subtraction
# (logits are O(1) at this problem's scale). Partition-broadcasts (rsqrt of
# rmsnorm, softmax row-normalizers) are rank-1 ones-matmuls on the tensor
# engine instead of DRAM round-trips.

import numpy as np

B, T, D, H, Dh, F = 2, 2048, 1024, 16, 64, 4096
BT = B * T
NC_ = 8
TLOC = BT // NC_          # 512 tokens per core (256 of each batch)
SUB = 256                 # tokens per (core, batch)
EPS = 1e-6

N_AG = 128 * 8 * SUB      # one AllGather shard: [128 p, 8 dk, 256 t] fp8


def _patch_tile_tail():
    # This container's walrus rejects the InstISA sem_clear/dma_reset that
    # TileContext emits at kernel tail ("ISA wrong length"). The clears only
    # matter for re-executing a loaded NEFF; skip emitting them and keep the
    # bookkeeping.
    import concourse.bass as bass
    if getattr(bass.Bass, "_acfs_patched", False):
        return
    def _cfs(self, sems):
        if not sems:
            return
        sem_nums = [s.num if hasattr(s, "num") else s for s in sems]
        self._state.prepend_free_semaphores(sem_nums)
        for poison_set in self._tile_sem_poison_stack:
            poison_set.update(sem_nums)
    bass.Bass.clear_and_free_semaphores = _cfs
    bass.Bass._acfs_patched = True


def _legalize_multiwaits(nc):
    # This container's walrus encodes at most ONE semaphore wait per
    # instruction. Tile attaches several. Split: hoist all but the last wait
    # into standalone single-wait EventSemaphore instructions on the same
    # engine, immediately before the original instruction (same block, so
    # per-engine program order is preserved).
    import concourse.mybir as mybir
    wid = 0
    for bb in nc.main_func.blocks:
        il = bb.instructions
        new_list = []
        for inst in il:
            si = getattr(inst, "sync_info", None)
            if si is not None and si.on_wait is not None and len(si.on_wait) > 1:
                waits = list(si.on_wait)
                for w in waits[:-1]:
                    ev = mybir.InstEventSemaphore(name=f"WSPLIT-{wid}", ins=[], outs=[])
                    wid += 1
                    ev.engine = inst.engine
                    ev.sync_info = mybir.SyncInfo(on_wait=[w], on_update=[])
                    new_list.append(ev)
                inst.sync_info = mybir.SyncInfo(on_wait=[waits[-1]],
                                                on_update=list(si.on_update))
            new_list.append(inst)
        il[:] = new_list


def _build(scales):
    import concourse.bass as bass
    import concourse.mybir as mybir
    import concourse.tile as tile
    from concourse.masks import make_identity

    _patch_tile_tail()

    f32 = mybir.dt.float32
    bf16 = mybir.dt.bfloat16
    f8 = mybir.dt.float8e4
    AF = mybir.ActivationFunctionType
    ALU = mybir.AluOpType
    SQ, SO, SG, SU, SD = (float(scales[k]) for k in ("qkv", "out", "gate", "up", "down"))
    DR = mybir.MatmulPerfMode.DoubleRow

    nc = bass.Bass(num_devices=NC_)
    RG = [list(range(NC_))]

    # ---- I/O (fp8 weights are host-side ternary-quantized, pre-transposed) ----
    x_slice = nc.dram_tensor("x_slice", [TLOC, D], f32, kind="ExternalInput")
    wqkvT = nc.dram_tensor("wqkvT", [128, 3 * 8 * 128], f8, kind="ExternalInput")
    woT = nc.dram_tensor("woT", [128, 8 * 1024], f8, kind="ExternalInput")
    wgT = nc.dram_tensor("wgT", [D, F], f8, kind="ExternalInput")
    wuT = nc.dram_tensor("wuT", [D, F], f8, kind="ExternalInput")
    wdT = nc.dram_tensor("wdT", [F, D], f8, kind="ExternalInput")
    out_d = nc.dram_tensor("out", [TLOC, D], f32, kind="ExternalOutput")

    def ap(t, off, dims):
        return bass.AP(tensor=t.tensor, offset=t.offset + off, ap=dims)

    with tile.TileContext(nc) as tc:
        import contextlib
        ctx = contextlib.ExitStack()
        with ctx:
            dram = ctx.enter_context(tc.tile_pool(name="dram", bufs=1, space="DRAM"))
            sing = ctx.enter_context(tc.tile_pool(name="sing", bufs=1))
            psS_p = ctx.enter_context(tc.tile_pool(name="psS", bufs=2, space="PSUM"))
            psO_p = ctx.enter_context(tc.tile_pool(name="psO", bufs=2, space="PSUM"))
            psA_p = ctx.enter_context(tc.tile_pool(name="psA", bufs=2, space="PSUM"))
            pool = ctx.enter_context(tc.tile_pool(name="pool", bufs=2))

            # ---- DRAM internals ----
            ag_in = [dram.tile([N_AG], f8, name=f"ag{i}_in") for i in range(2)]
            ag_out = [dram.tile([NC_ * N_AG], f8, name=f"ag{i}_out",
                                addr_space="Shared") for i in range(2)]
            a2a_in = [dram.tile([NC_, 128, SUB], f8, name=f"a2a{i}_in")
                      for i in range(2)]
            a2a_out = [dram.tile([NC_, 128, SUB], f8, name=f"a2a{i}_out")
                       for i in range(2)]

            # ---- persistent SBUF ----
            id_bf = sing.tile([128, 128], bf16, name="id_bf")
            make_identity(nc, id_bf)
            id_f32 = sing.tile([128, 128], f32, name="id_f32")
            make_identity(nc, id_f32)
            ones_col = sing.tile([128, 1], bf16, name="ones_col")
            nc.vector.memset(ones_col, 1.0)
            # ones at base partition 64 (bf16) for the softmax-denominator
            # broadcast matmul, whose rhs lives on partition 64
            ones65 = sing.tile([65, 128], bf16, name="ones65")
            nc.vector.memset(ones65, 1.0)
            # causal keep-mask M[p, u] = 1.0 iff p <= u - 384   (bf16, [128, 1024])
            mask_big = sing.tile([128, 1024], bf16, name="mask_big")
            nc.gpsimd.memset(mask_big, 1.0)
            nc.gpsimd.affine_select(
                out=mask_big, in_=mask_big, compare_op=ALU.is_ge, fill=0.0,
                base=-384, channel_multiplier=-1, pattern=[[1, 1024]],
            )
            eps_t = sing.tile([128, 1], f32, name="eps_t")
            nc.vector.memset(eps_t, EPS)

            wqkv_sb = sing.tile([128, 3, 8, 128], f8, name="wqkv_sb")
            wo_sb = sing.tile([128, 8, 1024], f8, name="wo_sb")
            qk_sb = sing.tile([128, 2, BT], bf16, name="qk_sb")   # q,k feature-major
            v_tm = sing.tile([128, 4, 16, 65], bf16, name="v_tm")  # per (b,hl): token-major v + ones col
            nc.vector.memset(v_tm[:, :, :, 64:65], 1.0)
            x_fm = sing.tile([128, 8, TLOC], f32, name="x_fm")     # residual stream, feature-major
            x2n = sing.tile([128, 8, TLOC], f8, name="x2n")
            a2a_sb = sing.tile([128, 8, TLOC], f8, name="a2a_sb")
            rstd1 = sing.tile([1, TLOC], f32, name="rstd1")
            rstd2 = sing.tile([1, TLOC], f32, name="rstd2")

            # ============ Stage A: x slice -> feature-major, rmsnorm1, AG ============
            # processed per batch-half so the first AllGather fires ASAP
            xh_fm = pool.tile([128, 2, 8, SUB], f8, name="xh_fm", tag="xhout", bufs=1)
            for bh in range(2):
                lo = bh * SUB
                for tt in range(2):
                    xs = pool.tile([128, 1024], f32, name="xs", tag="raw4", bufs=2)
                    nc.sync.dma_start(
                        out=xs, in_=x_slice[lo + tt * 128: lo + (tt + 1) * 128, :])
                    for dkq in range(2):
                        ps = psA_p.tile([128, 512], f32, name="psA", tag="psA")
                        for kk in range(4):
                            dk = dkq * 4 + kk
                            nc.tensor.transpose(ps[:, kk * 128:(kk + 1) * 128],
                                                xs[:, dk * 128:(dk + 1) * 128], id_f32)
                        nc.vector.tensor_copy(
                            x_fm[:, dkq * 4:(dkq + 1) * 4, lo + tt * 128: lo + (tt + 1) * 128],
                            ps.rearrange("p (a b) -> p a b", b=128))
                psn = psA_p.tile([1, SUB], f32, name="psA", tag="psA")
                for m in range(8):
                    sq = pool.tile([128, SUB], bf16, name="sq", tag="sqb", bufs=3)
                    nc.vector.tensor_mul(sq, x_fm[:, m, lo:lo + SUB],
                                         x_fm[:, m, lo:lo + SUB])
                    nc.tensor.matmul(psn, ones_col, sq, start=(m == 0), stop=(m == 7))
                # sqrt -> bf16, broadcast via ones-matmul, reciprocal of the
                # broadcast (vector-parallel over 128 lanes), then scale
                r1 = rstd1[:, lo:lo + SUB]
                nc.scalar.activation(r1, psn, AF.Sqrt, scale=1.0 / D,
                                     bias=eps_t[0:1, :])
                r1b = pool.tile([1, SUB], bf16, name="r1b", tag="r1b", bufs=2)
                nc.vector.tensor_copy(r1b, r1)
                psb = psA_p.tile([128, SUB], f32, name="psA", tag="psA")
                nc.tensor.matmul(psb, ones65[0:1, :], r1b, start=True, stop=True)
                rb = pool.tile([128, SUB], f32, name="rb", tag="rb", bufs=2)
                nc.vector.reciprocal(rb, psb)
                for m in range(8):
                    nc.vector.tensor_mul(xh_fm[:, bh, m, :],
                                         x_fm[:, m, lo:lo + SUB], rb)
                nc.sync.dma_start(
                    out=ap(ag_in[bh], 0, [[8 * SUB, 128], [1, 8 * SUB]]),
                    in_=xh_fm[:, bh, :, :].rearrange("p a b -> p (a b)"))
                nc.gpsimd.collective_compute(
                    "AllGather", ALU.bypass, replica_groups=RG,
                    ins=[ag_in[bh][:].opt()], outs=[ag_out[bh][:].opt()])

            # weight loads (pure DMA; no on-device quantization needed)
            nc.sync.dma_start(out=wqkv_sb.rearrange("p a b c -> p (a b c)"),
                              in_=wqkvT[:, :])
            nc.sync.dma_start(out=wo_sb.rearrange("p a b -> p (a b)"), in_=woT[:, :])

            # ============ qkv (per AG chunk) + attention, interleaved ============
            def qkv_chunk(bh, r):
                # chunk r of AG bh: xhat feature-major for batch-bh tokens
                # [256r, 256r+256); produce q/k (feature-major) + v (token-major)
                xh_sb = pool.tile([128, 8, SUB], f8, name="xh_sb", tag="xh8",
                                  bufs=3)
                nc.sync.dma_start(
                    out=xh_sb.rearrange("p a b -> p (a b)"),
                    in_=ap(ag_out[bh], r * N_AG, [[8 * SUB, 128], [1, 8 * SUB]]))
                for m in range(3):
                    ps = psA_p.tile([128, SUB], f32, name="psA", tag="psA")
                    for a in range(4):
                        nc.tensor.matmul(
                            ps, wqkv_sb[:, m, 2 * a:2 * a + 2, :],
                            xh_sb[:, 2 * a:2 * a + 2, :],
                            start=(a == 0), stop=(a == 3), perf_mode=DR)
                    if m < 2:
                        nc.vector.tensor_copy(
                            qk_sb[:, m, bh * 2048 + r * SUB: bh * 2048 + (r + 1) * SUB],
                            ps)
                    else:
                        v_ch = pool.tile([128, SUB], bf16, name="v_ch", tag="vch", bufs=2)
                        nc.scalar.copy(v_ch, ps)
                        psv = psA_p.tile([128, SUB], bf16, name="psA", tag="psA")
                        for jj in range(2):
                            nc.tensor.transpose(psv[:, jj * 128:(jj + 1) * 128],
                                                v_ch[:, jj * 128:(jj + 1) * 128], id_bf)
                        psv_j = psv.rearrange("p (j x) -> p j x", x=128)
                        for hl in range(2):
                            # psv cols: [jj 2 x (hl0 64 | hl1 64)] -> v_tm j 2r, 2r+1
                            nc.vector.tensor_copy(
                                v_tm[:, bh * 2 + hl, 2 * r:2 * r + 2, 0:64],
                                psv_j[:, :, hl * 64:hl * 64 + 64])

            def attention_n(b, n):
                q0 = qk_sb[0:64, 0, b * 2048:(b + 1) * 2048]
                q1 = qk_sb[64:128, 0, b * 2048:(b + 1) * 2048]
                k0 = qk_sb[0:64, 1, b * 2048:(b + 1) * 2048]
                k1 = qk_sb[64:128, 1, b * 2048:(b + 1) * 2048]
                pso = [psO_p.tile([65, 512], f32, name="psO", tag="psO")
                       for _ in range(2)]
                jmax = 4 * n + 3
                av_q = []
                for j in range(jmax + 1):
                    pss = psS_p.tile([128, 1024], f32, name="psS", tag="psS")
                    nc.tensor.matmul(pss[:, 0:512], k0[:, j * 128:(j + 1) * 128],
                                     q0[:, n * 512:(n + 1) * 512], start=True, stop=True)
                    nc.tensor.matmul(pss[:, 512:1024], k1[:, j * 128:(j + 1) * 128],
                                     q1[:, n * 512:(n + 1) * 512], start=True, stop=True)
                    # emit the previous iteration's AV here so it sits behind
                    # this QK in the tensor queue; its exp/mask ran meanwhile
                    if len(av_q) >= 1:
                        av_q.pop(0)()
                    pt = pool.tile([128, 1024], bf16, name="pt", tag="pt", bufs=4)
                    nc.scalar.activation(pt, pss, AF.Exp, scale=SQ * SQ * (Dh ** -0.5))
                    if j >= 4 * n:
                        off = 512 * n - 128 * j + 384
                        nc.gpsimd.tensor_mul(pt[:, 0:512], pt[:, 0:512],
                                             mask_big[:, off:off + 512])
                        nc.gpsimd.tensor_mul(pt[:, 512:1024], pt[:, 512:1024],
                                             mask_big[:, off:off + 512])

                    def mk_av(j, pt):
                        def go():
                            for hl in range(2):
                                nc.tensor.matmul(
                                    pso[hl], v_tm[:, b * 2 + hl, j, :],
                                    pt[:, hl * 512:(hl + 1) * 512],
                                    start=(j == 0), stop=(j == jmax))
                        return go
                    av_q.append(mk_av(j, pt))
                for go in av_q:
                    go()
                # evict; cast the rowsum row to bf16 on partition 64. The
                # broadcast matmul then depends only on the cheap cast; the
                # expensive per-element division happens on vector via a
                # fused divide, emitted one n-block later (finisher).
                tails = []
                for hl in range(2):
                    o_sb = pool.tile([65, 512], f32, name="o_sb", tag="osb", bufs=5)
                    nc.scalar.copy(o_sb, pso[hl])
                    # 1/rowsum = exp(-ln(x)) on scalar, straight to bf16
                    lnr = pool.tile([65, 512], f32, name="lnr", tag="lnr", bufs=3)
                    nc.scalar.activation(lnr[64:65, :], o_sb[64:65, :], AF.Ln)
                    rsbf = pool.tile([65, 512], bf16, name="rsbf", tag="rsbf", bufs=5)
                    nc.scalar.activation(rsbf[64:65, :], lnr[64:65, :], AF.Exp,
                                         scale=-1.0)
                    tails.append((o_sb, rsbf))

                def finish():
                    for hl, (o_sb, rsbf) in enumerate(tails):
                        psb2 = psA_p.tile([64, 512], f32, name="psA", tag="psA")
                        nc.tensor.matmul(psb2, ones65[64:65, 0:64], rsbf[64:65, :],
                                         start=True, stop=True)
                        o_n = pool.tile([64, 512], f8, name="o_n", tag="on", bufs=3)
                        nc.vector.scalar_tensor_tensor(
                            out=o_n, in0=o_sb[0:64, :], scalar=SQ,
                            op0=ALU.mult, op1=ALU.mult, in1=psb2)
                        for half in range(2):
                            nc.sync.dma_start(
                                out=ap(a2a_in[b],
                                       (2 * n + half) * 128 * SUB + hl * 64 * SUB,
                                       [[SUB, 64], [1, SUB]]),
                                in_=o_n[:, half * SUB:(half + 1) * SUB])
                return finish

            def outproj_norm2(b):
                lo, hi = b * SUB, (b + 1) * SUB
                for m in range(8):
                    ps = psA_p.tile([128, SUB], f32, name="psA", tag="psA")
                    for a in range(4):
                        nc.tensor.matmul(
                            ps, wo_sb[:, 2 * a:2 * a + 2, m * 128:(m + 1) * 128],
                            a2a_sb[:, 2 * a:2 * a + 2, lo:hi],
                            start=(a == 0), stop=(a == 3), perf_mode=DR)
                    nc.vector.scalar_tensor_tensor(
                        out=x_fm[:, m, lo:hi], in0=ps, scalar=SO, op0=ALU.mult,
                        op1=ALU.add, in1=x_fm[:, m, lo:hi])
                psn2 = psA_p.tile([1, SUB], f32, name="psA", tag="psA")
                for m in range(8):
                    sq2 = pool.tile([128, SUB], bf16, name="sq2", tag="sq2", bufs=3)
                    nc.vector.tensor_mul(sq2, x_fm[:, m, lo:hi], x_fm[:, m, lo:hi])
                    nc.tensor.matmul(psn2, ones_col, sq2, start=(m == 0), stop=(m == 7))
                r2 = rstd2[:, lo:hi]
                nc.scalar.activation(r2, psn2, AF.Sqrt, scale=1.0 / D,
                                     bias=eps_t[0:1, :])
                r2b = pool.tile([1, SUB], bf16, name="r2b", tag="r1b", bufs=2)
                nc.vector.tensor_copy(r2b, r2)
                psb3 = psA_p.tile([128, SUB], f32, name="psA", tag="psA")
                nc.tensor.matmul(psb3, ones65[0:1, :], r2b, start=True, stop=True)
                rb3 = pool.tile([128, SUB], f32, name="rb3", tag="rb", bufs=2)
                nc.vector.reciprocal(rb3, psb3)
                for m in range(8):
                    nc.vector.tensor_mul(x2n[:, m, lo:hi], x_fm[:, m, lo:hi], rb3)

            def fire_a2a(b):
                nc.gpsimd.collective_compute(
                    "AllToAll", ALU.bypass, replica_groups=RG,
                    ins=[a2a_in[b][:].opt()], outs=[a2a_out[b][:].opt()])
                nc.sync.dma_start(out=a2a_sb[:, :, b * SUB:(b + 1) * SUB],
                                  in_=a2a_out[b].rearrange("r p t -> p r t"))

            def load_gu(half, gl):
                gf0 = half * 16 + gl * 8
                wg_t = pool.tile([128, 8, 1024], f8, name="wg_t", tag="w8", bufs=4)
                for dk in range(8):
                    nc.sync.dma_start(
                        out=wg_t[:, dk, :],
                        in_=wgT[dk * 128:(dk + 1) * 128, gf0 * 128:gf0 * 128 + 1024])
                wu_t = pool.tile([128, 8, 1024], f8, name="wu_t", tag="w8", bufs=4)
                for dk in range(8):
                    nc.sync.dma_start(
                        out=wu_t[:, dk, :],
                        in_=wuT[dk * 128:(dk + 1) * 128, gf0 * 128:gf0 * 128 + 1024])
                return wg_t, wu_t

            def gu_block(wg_t, wu_t, a_sb, gl, c0, c1):
                # F-tiles gl*8..gl*8+8 of the half, token cols [c0, c1)
                for q2 in range(2):
                    sgs = []
                    for mm in range(4):
                        fi = q2 * 4 + mm
                        psg = psA_p.tile([128, c1 - c0], f32, name="psA", tag="psA")
                        for a in range(4):
                            nc.tensor.matmul(
                                psg, wg_t[:, 2 * a:2 * a + 2, fi * 128:(fi + 1) * 128],
                                x2n[:, 2 * a:2 * a + 2, c0:c1],
                                start=(a == 0), stop=(a == 3), perf_mode=DR)
                        sg = pool.tile([128, c1 - c0], bf16, name="sg", tag="sg", bufs=5)
                        nc.scalar.activation(sg, psg, AF.Silu, scale=SG)
                        sgs.append(sg)
                    for mm in range(4):
                        fi = q2 * 4 + mm
                        psu = psA_p.tile([128, c1 - c0], f32, name="psA", tag="psA")
                        for a in range(4):
                            nc.tensor.matmul(
                                psu, wu_t[:, 2 * a:2 * a + 2, fi * 128:(fi + 1) * 128],
                                x2n[:, 2 * a:2 * a + 2, c0:c1],
                                start=(a == 0), stop=(a == 3), perf_mode=DR)
                        nc.vector.scalar_tensor_tensor(
                            out=a_sb[:, gl * 8 + fi, c0:c1], in0=psu, scalar=SU,
                            op0=ALU.mult, op1=ALU.mult, in1=sgs[mm])

            # --- batch 0: qkv + attention (normalization tails pipelined) ---
            pend = None
            for n in range(4):
                qkv_chunk(0, 2 * n)
                qkv_chunk(0, 2 * n + 1)
                fin = attention_n(0, n)
                if pend:
                    pend()
                pend = fin
            pend()
            fire_a2a(0)
            # --- batch 1 attention; batch-0 out_proj/norm2 overlaps its middle ---
            pend = None
            for n in range(4):
                qkv_chunk(1, 2 * n)
                qkv_chunk(1, 2 * n + 1)
                fin = attention_n(1, n)
                if pend:
                    pend()
                pend = fin
                if n == 1:
                    outproj_norm2(0)
            # prefetch first FFN gate/up weights (DMA only), finish the last
            # attention tail, fire the second AllToAll, then fill its window
            # with the first gate/up block on batch-0 tokens
            a_sb0 = pool.tile([128, 16, TLOC], bf16, name="a_sb", tag="asb", bufs=2)
            wg0, wu0 = load_gu(0, 0)
            pend()
            fire_a2a(1)
            gu_block(wg0, wu0, a_sb0, 0, 0, SUB)
            outproj_norm2(1)

            # ============ FFN (two F-halves; down accumulated into x_fm) ============
            for half in range(2):
                a_sb = a_sb0 if half == 0 else pool.tile(
                    [128, 16, TLOC], bf16, name="a_sb", tag="asb", bufs=2)
                for gl in range(2):
                    if half == 0 and gl == 0:
                        gu_block(wg0, wu0, a_sb, 0, SUB, TLOC)
                        continue
                    wg_t, wu_t = load_gu(half, gl)
                    gu_block(wg_t, wu_t, a_sb, gl, 0, TLOC)
                wds = []
                for dh in range(2):
                    wd_t = pool.tile([128, 8, 1024], f8, name="wd_t", tag="w8", bufs=4)
                    for f8_ in range(8):
                        fk = half * 16 + dh * 8 + f8_
                        nc.sync.dma_start(out=wd_t[:, f8_, :],
                                          in_=wdT[fk * 128:(fk + 1) * 128, :])
                    wds.append(wd_t)
                for mo in range(8):
                    psd = psA_p.tile([128, 512], f32, name="psA", tag="psA")
                    for ff in range(16):
                        nc.tensor.matmul(
                            psd, wds[ff // 8][:, ff % 8, mo * 128:(mo + 1) * 128],
                            a_sb[:, ff, :], start=(ff == 0), stop=(ff == 15))
                    nc.vector.scalar_tensor_tensor(
                        out=x_fm[:, mo, :], in0=psd, scalar=SD, op0=ALU.mult,
                        op1=ALU.add, in1=x_fm[:, mo, :])
                    # once a 4-tile do-group is final (second F-half), emit its
                    # output transposes + stores immediately
                    if half == 1 and mo % 4 == 3:
                        doh = mo // 4
                        for tt in range(4):
                            ps = psA_p.tile([128, 512], f32, name="psA", tag="psA")
                            for mm in range(4):
                                nc.tensor.transpose(
                                    ps[:, mm * 128:(mm + 1) * 128],
                                    x_fm[:, doh * 4 + mm, tt * 128:(tt + 1) * 128],
                                    id_f32)
                            ob = pool.tile([128, 512], f32, name="ob", tag="f2k",
                                           bufs=3)
                            nc.vector.tensor_copy(ob, ps)
                            nc.sync.dma_start(
                                out=out_d[tt * 128:(tt + 1) * 128,
                                          doh * 512:(doh + 1) * 512],
                                in_=ob)
    _legalize_multiwaits(nc)
    return nc


def _quant(w, s):
    # host-side ternary quantization (exact in fp8-e4m3); matches
    # jnp.clip(jnp.round(W/s), -1, 1) including round-half-to-even
    return np.clip(np.round(w / s), -1.0, 1.0).astype(np.float32)


def _prepare(inputs):
    import ml_dtypes
    f8 = ml_dtypes.float8_e4m3
    x = np.asarray(inputs["x"], np.float32).reshape(BT, D)
    qkv_w = np.asarray(inputs["qkv_w"], np.float32)
    out_w = np.asarray(inputs["out_w"], np.float32)
    gate_w = np.asarray(inputs["gate_w"], np.float32)
    up_w = np.asarray(inputs["up_w"], np.float32)
    down_w = np.asarray(inputs["down_w"], np.float32)
    ln1 = np.asarray(inputs["ln1_w"], np.float32)
    ln2 = np.asarray(inputs["ln2_w"], np.float32)

    scales = {
        "qkv": max(np.mean(np.abs(qkv_w), dtype=np.float32), np.float32(1e-5)),
        "out": max(np.mean(np.abs(out_w), dtype=np.float32), np.float32(1e-5)),
        "gate": max(np.mean(np.abs(gate_w), dtype=np.float32), np.float32(1e-5)),
        "up": max(np.mean(np.abs(up_w), dtype=np.float32), np.float32(1e-5)),
        "down": max(np.mean(np.abs(down_w), dtype=np.float32), np.float32(1e-5)),
    }
    qkv_q = _quant(qkv_w, scales["qkv"]) * ln1[None, :]     # fold g1 into cols (d)
    out_q = _quant(out_w, scales["out"])
    gate_q = _quant(gate_w, scales["gate"]) * ln2[None, :]  # fold g2 into cols (d)
    up_q = _quant(up_w, scales["up"]) * ln2[None, :]
    down_q = _quant(down_w, scales["down"])

    # shared (identical on all cores), matmul-ready transposed layouts
    # woT[p, r, do] = out_q[do, r*128+p]
    woT = np.ascontiguousarray(
        out_q.T.reshape(8, 128, 1024).transpose(1, 0, 2).reshape(128, 8192)
    ).astype(f8)
    wgT = np.ascontiguousarray(gate_q.T).astype(f8)         # [1024 d, 4096 F]
    wuT = np.ascontiguousarray(up_q.T).astype(f8)
    wdT = np.ascontiguousarray(down_q.T).astype(f8)         # [4096 F, 1024 do]

    in_maps = []
    for c in range(NC_):
        # wqkvT[p, m, dk, f] = qkv_q[m*1024 + c*128 + f, dk*128 + p]
        wq = np.stack([
            qkv_q[m * 1024 + c * 128: m * 1024 + (c + 1) * 128, :]
            .T.reshape(8, 128, 128).transpose(1, 0, 2)
            for m in range(3)], axis=1)                      # [128, 3, 8, 128]
        in_maps.append({
            "x_slice": np.ascontiguousarray(np.concatenate(
                [x[SUB * c: SUB * (c + 1)],
                 x[T + SUB * c: T + SUB * (c + 1)]], axis=0)),
            "wqkvT": np.ascontiguousarray(wq.reshape(128, 3072)).astype(f8),
            "woT": woT,
            "wgT": wgT,
            "wuT": wuT,
            "wdT": wdT,
        })
    return scales, in_maps


def run(inputs, trace=False):
    from concourse.bass_utils import run_bass_kernel_spmd
    scales, in_maps = _prepare(inputs)
    nc = _build(scales)
    res = run_bass_kernel_spmd(nc, in_maps, list(range(NC_)), trace=trace)
    out = np.empty((BT, D), np.float32)
    for c in range(NC_):
        o = np.asarray(res.results[c]["out"])
        out[SUB * c: SUB * (c + 1)] = o[0:SUB]
        out[T + SUB * c: T + SUB * (c + 1)] = o[SUB:2 * SUB]
    return out.reshape(B, T, D), res


def kernel(**inputs):
    out, _ = run(inputs, trace=False)
    return out


# revision 40
# speedup vs baseline: 1.2373x; 1.1621x over previous
# Trainium2 Bass kernel for a BitLinear transformer block (attention + SwiGLU FFN).
#
# Sharding across 8 NeuronCores (hybrid):
#   - Attention: head-parallel. Core c computes q/k/v + causal attention for
#     global heads {2c, 2c+1}, both batches, over ALL tokens.
#   - out_proj / rmsnorm2 / FFN: token-parallel. Core c handles 256 tokens of
#     batch 0 ([256c, 256c+256)) and the same range of batch 1, with the FULL
#     weight matrices.
#   - Collectives: two half AllGathers of the rmsnorm'd activations (computed
#     sequence-parallel, shipped feature-major in fp8), and two AllToAlls (one
#     per batch, fp8) that re-shard attention outputs from head-parallel to
#     token-parallel. The batch splits let batch-0's collective and out_proj
#     overlap batch-1's attention.
#
# Weights are pre-quantized on the host to ternary {-1,0,1} in matmul-ready
# transposed layouts; ternary values are exact in fp8-e4m3, so the qkv /
# out_proj / gate / up / down weights ship as fp8 (half the HBM traffic of
# bf16 at identical matmul speed). Activations on the quantized-matmul paths
# (xhat, attention out, x2n) are fp8 as well; q/k/v, softmax, and the SwiGLU
# product stay bf16; all matmuls accumulate in fp32 PSUM and the residual
# stream stays fp32 end-to-end. Softmax runs in fp32 without max-subtraction
# (logits are O(1) at this problem's scale). Partition-broadcasts (rsqrt of
# rmsnorm, softmax row-normalizers) are rank-1 ones-matmuls on the tensor
# engine instead of DRAM round-trips.

import numpy as np

B, T, D, H, Dh, F = 2, 2048, 1024, 16, 64, 4096
BT = B * T
NC_ = 8
TLOC = BT // NC_          # 512 tokens per core (256 of each batch)
SUB = 256                 # tokens per (core, batch)
EPS = 1e-6

N_AG = 128 * 8 * SUB      # one AllGather shard: [128 p, 8 dk, 256 t] fp8


def _patch_tile_tail():
    # This container's walrus rejects the InstISA sem_clear/dma_reset that
    # TileContext emits at kernel tail ("ISA wrong length"). The clears only
    # matter for re-executing a loaded NEFF; skip emitting them and keep the
    # bookkeeping.
    import concourse.bass as bass
    if getattr(bass.Bass, "_acfs_patched", False):
        return
    def _cfs(self, sems):
        if not sems:
            return
        sem_nums = [s.num if hasattr(s, "num") else s for s in sems]
        self._state.prepend_free_semaphores(sem_nums)
        for poison_set in self._tile_sem_poison_stack:
            poison_set.update(sem_nums)
    bass.Bass.clear_and_free_semaphores = _cfs
    bass.Bass._acfs_patched = True


def _legalize_multiwaits(nc):
    # This container's walrus encodes at most ONE semaphore wait per
    # instruction. Tile attaches several. Split: hoist all but the last wait
    # into standalone single-wait EventSemaphore instructions on the same
    # engine, immediately before the original instruction (same block, so
    # per-engine program order is preserved).
    import concourse.mybir as mybir
    wid = 0
    for bb in nc.main_func.blocks:
        il = bb.instructions
        new_list = []
        for inst in il:
            si = getattr(inst, "sync_info", None)
            if si is not None and si.on_wait is not None and len(si.on_wait) > 1:
                waits = list(si.on_wait)
                for w in waits[:-1]:
                    ev = mybir.InstEventSemaphore(name=f"WSPLIT-{wid}", ins=[], outs=[])
                    wid += 1
                    ev.engine = inst.engine
                    ev.sync_info = mybir.SyncInfo(on_wait=[w], on_update=[])
                    new_list.append(ev)
                inst.sync_info = mybir.SyncInfo(on_wait=[waits[-1]],
                                                on_update=list(si.on_update))
            new_list.append(inst)
        il[:] = new_list


def _build(scales):
    import concourse.bass as bass
    import concourse.mybir as mybir
    import concourse.tile as tile
    from concourse.masks import make_identity

    _patch_tile_tail()

    f32 = mybir.dt.float32
    bf16 = mybir.dt.bfloat16
    f8 = mybir.dt.float8e4
    AF = mybir.ActivationFunctionType
    ALU = mybir.AluOpType
    SQ, SO, SG, SU, SD = (float(scales[k]) for k in ("qkv", "out", "gate", "up", "down"))
    DR = mybir.MatmulPerfMode.DoubleRow

    nc = bass.Bass(num_devices=NC_)
    RG = [list(range(NC_))]

    # ---- I/O (fp8 weights are host-side ternary-quantized, pre-transposed) ----
    x_slice = nc.dram_tensor("x_slice", [TLOC, D], f32, kind="ExternalInput")
    wqkvT = nc.dram_tensor("wqkvT", [128, 3 * 8 * 128], f8, kind="ExternalInput")
    woT = nc.dram_tensor("woT", [128, 8 * 1024], f8, kind="ExternalInput")
    wgT = nc.dram_tensor("wgT", [D, F], f8, kind="ExternalInput")
    wuT = nc.dram_tensor("wuT", [D, F], f8, kind="ExternalInput")
    wdT = nc.dram_tensor("wdT", [F, D], f8, kind="ExternalInput")
    out_d = nc.dram_tensor("out", [TLOC, D], f32, kind="ExternalOutput")

    def ap(t, off, dims):
        return bass.AP(tensor=t.tensor, offset=t.offset + off, ap=dims)

    with tile.TileContext(nc) as tc:
        import contextlib
        ctx = contextlib.ExitStack()
        with ctx:
            dram = ctx.enter_context(tc.tile_pool(name="dram", bufs=1, space="DRAM"))
            sing = ctx.enter_context(tc.tile_pool(name="sing", bufs=1))
            psS_p = ctx.enter_context(tc.tile_pool(name="psS", bufs=2, space="PSUM"))
            psO_p = ctx.enter_context(tc.tile_pool(name="psO", bufs=2, space="PSUM"))
            psA_p = ctx.enter_context(tc.tile_pool(name="psA", bufs=2, space="PSUM"))
            pool = ctx.enter_context(tc.tile_pool(name="pool", bufs=2))

            # ---- DRAM internals ----
            ag_in = [dram.tile([N_AG], f8, name=f"ag{i}_in") for i in range(2)]
            ag_out = [dram.tile([NC_ * N_AG], f8, name=f"ag{i}_out",
                                addr_space="Shared") for i in range(2)]
            a2a_in = [dram.tile([NC_, 128, SUB], f8, name=f"a2a{i}_in")
                      for i in range(2)]
            a2a_out = [dram.tile([NC_, 128, SUB], f8, name=f"a2a{i}_out")
                       for i in range(2)]

            # ---- persistent SBUF ----
            id_bf = sing.tile([128, 128], bf16, name="id_bf")
            make_identity(nc, id_bf)
            id_f32 = sing.tile([128, 128], f32, name="id_f32")
            make_identity(nc, id_f32)
            ones_col = sing.tile([128, 1], bf16, name="ones_col")
            nc.vector.memset(ones_col, 1.0)
            # ones at base partition 64 (bf16) for the softmax-denominator
            # broadcast matmul, whose rhs lives on partition 64
            ones65 = sing.tile([65, 128], bf16, name="ones65")
            nc.vector.memset(ones65, 1.0)
            # causal keep-mask M[p, u] = 1.0 iff p <= u - 384   (bf16, [128, 1024])
            mask_big = sing.tile([128, 1024], bf16, name="mask_big")
            nc.gpsimd.memset(mask_big, 1.0)
            nc.gpsimd.affine_select(
                out=mask_big, in_=mask_big, compare_op=ALU.is_ge, fill=0.0,
                base=-384, channel_multiplier=-1, pattern=[[1, 1024]],
            )
            eps_t = sing.tile([128, 1], f32, name="eps_t")
            nc.vector.memset(eps_t, EPS)

            wqkv_sb = sing.tile([128, 3, 8, 128], f8, name="wqkv_sb")
            wo_sb = sing.tile([128, 8, 1024], f8, name="wo_sb")
            qk_sb = sing.tile([128, 2, BT], bf16, name="qk_sb")   # q,k feature-major
            v_tm = sing.tile([128, 4, 16, 65], bf16, name="v_tm")  # per (b,hl): token-major v + ones col
            nc.vector.memset(v_tm[:, :, :, 64:65], 1.0)
            x_fm = sing.tile([128, 8, TLOC], f32, name="x_fm")     # residual stream, feature-major
            x2n = sing.tile([128, 8, TLOC], f8, name="x2n")
            a2a_sb = sing.tile([128, 8, TLOC], f8, name="a2a_sb")
            rstd1 = sing.tile([1, TLOC], f32, name="rstd1")
            rstd2 = sing.tile([1, TLOC], f32, name="rstd2")

            # ============ Stage A: x slice -> feature-major, rmsnorm1, AG ============
            # processed per batch-half so the first AllGather fires ASAP
            xh_fm = pool.tile([128, 2, 8, SUB], f8, name="xh_fm", tag="xhout", bufs=1)
            for bh in range(2):
                lo = bh * SUB
                for tt in range(2):
                    xs = pool.tile([128, 1024], f32, name="xs", tag="raw4", bufs=2)
                    nc.sync.dma_start(
                        out=xs, in_=x_slice[lo + tt * 128: lo + (tt + 1) * 128, :])
                    for dkq in range(2):
                        ps = psA_p.tile([128, 512], f32, name="psA", tag="psA")
                        for kk in range(4):
                            dk = dkq * 4 + kk
                            nc.tensor.transpose(ps[:, kk * 128:(kk + 1) * 128],
                                                xs[:, dk * 128:(dk + 1) * 128], id_f32)
                        nc.vector.tensor_copy(
                            x_fm[:, dkq * 4:(dkq + 1) * 4, lo + tt * 128: lo + (tt + 1) * 128],
                            ps.rearrange("p (a b) -> p a b", b=128))
                psn = psA_p.tile([1, SUB], f32, name="psA", tag="psA")
                for m in range(8):
                    sq = pool.tile([128, SUB], bf16, name="sq", tag="sqb", bufs=3)
                    nc.vector.tensor_mul(sq, x_fm[:, m, lo:lo + SUB],
                                         x_fm[:, m, lo:lo + SUB])
                    nc.tensor.matmul(psn, ones_col, sq, start=(m == 0), stop=(m == 7))
                # sqrt -> bf16, broadcast via ones-matmul, reciprocal of the
                # broadcast (vector-parallel over 128 lanes), then scale
                r1 = rstd1[:, lo:lo + SUB]
                nc.scalar.activation(r1, psn, AF.Sqrt, scale=1.0 / D,
                                     bias=eps_t[0:1, :])
                r1b = pool.tile([1, SUB], bf16, name="r1b", tag="r1b", bufs=2)
                nc.vector.tensor_copy(r1b, r1)
                psb = psA_p.tile([128, SUB], f32, name="psA", tag="psA")
                nc.tensor.matmul(psb, ones65[0:1, :], r1b, start=True, stop=True)
                rb = pool.tile([128, SUB], f32, name="rb", tag="rb", bufs=2)
                nc.vector.reciprocal(rb, psb)
                for m in range(8):
                    nc.vector.tensor_mul(xh_fm[:, bh, m, :],
                                         x_fm[:, m, lo:lo + SUB], rb)
                nc.sync.dma_start(
                    out=ap(ag_in[bh], 0, [[8 * SUB, 128], [1, 8 * SUB]]),
                    in_=xh_fm[:, bh, :, :].rearrange("p a b -> p (a b)"))
                nc.gpsimd.collective_compute(
                    "AllGather", ALU.bypass, replica_groups=RG,
                    ins=[ag_in[bh][:].opt()], outs=[ag_out[bh][:].opt()])

            # weight loads (pure DMA; no on-device quantization needed)
            nc.sync.dma_start(out=wqkv_sb.rearrange("p a b c -> p (a b c)"),
                              in_=wqkvT[:, :])
            nc.sync.dma_start(out=wo_sb.rearrange("p a b -> p (a b)"), in_=woT[:, :])

            # ============ qkv (per AG chunk) + attention, interleaved ============
            def qkv_chunk(bh, r):
                # chunk r of AG bh: xhat feature-major for batch-bh tokens
                # [256r, 256r+256); produce q/k (feature-major) + v (token-major)
                xh_sb = pool.tile([128, 8, SUB], f8, name="xh_sb", tag="xh8",
                                  bufs=3)
                nc.sync.dma_start(
                    out=xh_sb.rearrange("p a b -> p (a b)"),
                    in_=ap(ag_out[bh], r * N_AG, [[8 * SUB, 128], [1, 8 * SUB]]))
                for m in range(3):
                    ps = psA_p.tile([128, SUB], f32, name="psA", tag="psA")
                    for a in range(4):
                        nc.tensor.matmul(
                            ps, wqkv_sb[:, m, 2 * a:2 * a + 2, :],
                            xh_sb[:, 2 * a:2 * a + 2, :],
                            start=(a == 0), stop=(a == 3), perf_mode=DR)
                    if m < 2:
                        nc.vector.tensor_copy(
                            qk_sb[:, m, bh * 2048 + r * SUB: bh * 2048 + (r + 1) * SUB],
                            ps)
                    else:
                        v_ch = pool.tile([128, SUB], bf16, name="v_ch", tag="vch", bufs=2)
                        nc.scalar.copy(v_ch, ps)
                        psv = psA_p.tile([128, SUB], bf16, name="psA", tag="psA")
                        for jj in range(2):
                            nc.tensor.transpose(psv[:, jj * 128:(jj + 1) * 128],
                                                v_ch[:, jj * 128:(jj + 1) * 128], id_bf)
                        psv_j = psv.rearrange("p (j x) -> p j x", x=128)
                        for hl in range(2):
                            # psv cols: [jj 2 x (hl0 64 | hl1 64)] -> v_tm j 2r, 2r+1
                            nc.vector.tensor_copy(
                                v_tm[:, bh * 2 + hl, 2 * r:2 * r + 2, 0:64],
                                psv_j[:, :, hl * 64:hl * 64 + 64])

            def attention_n(b, n):
                q0 = qk_sb[0:64, 0, b * 2048:(b + 1) * 2048]
                q1 = qk_sb[64:128, 0, b * 2048:(b + 1) * 2048]
                k0 = qk_sb[0:64, 1, b * 2048:(b + 1) * 2048]
                k1 = qk_sb[64:128, 1, b * 2048:(b + 1) * 2048]
                pso = [psO_p.tile([65, 512], f32, name="psO", tag="psO")
                       for _ in range(2)]
                jmax = 4 * n + 3
                av_q = []
                for j in range(jmax + 1):
                    pss = psS_p.tile([128, 1024], f32, name="psS", tag="psS")
                    nc.tensor.matmul(pss[:, 0:512], k0[:, j * 128:(j + 1) * 128],
                                     q0[:, n * 512:(n + 1) * 512], start=True, stop=True)
                    nc.tensor.matmul(pss[:, 512:1024], k1[:, j * 128:(j + 1) * 128],
                                     q1[:, n * 512:(n + 1) * 512], start=True, stop=True)
                    # emit the previous iteration's AV here so it sits behind
                    # this QK in the tensor queue; its exp/mask ran meanwhile
                    if len(av_q) >= 1:
                        av_q.pop(0)()
                    pt = pool.tile([128, 1024], bf16, name="pt", tag="pt", bufs=4)
                    nc.scalar.activation(pt, pss, AF.Exp, scale=SQ * SQ * (Dh ** -0.5))
                    if j >= 4 * n:
                        off = 512 * n - 128 * j + 384
                        nc.gpsimd.tensor_mul(pt[:, 0:512], pt[:, 0:512],
                                             mask_big[:, off:off + 512])
                        nc.gpsimd.tensor_mul(pt[:, 512:1024], pt[:, 512:1024],
                                             mask_big[:, off:off + 512])

                    def mk_av(j, pt):
                        def go():
                            for hl in range(2):
                                nc.tensor.matmul(
                                    pso[hl], v_tm[:, b * 2 + hl, j, :],
                                    pt[:, hl * 512:(hl + 1) * 512],
                                    start=(j == 0), stop=(j == jmax))
                        return go
                    av_q.append(mk_av(j, pt))
                for go in av_q:
                    go()
                # evict; cast the rowsum row to bf16 on partition 64. The
                # broadcast matmul then depends only on the cheap cast; the
                # expensive per-element division happens on vector via a
                # fused divide, emitted one n-block later (finisher).
                tails = []
                for hl in range(2):
                    o_sb = pool.tile([65, 512], f32, name="o_sb", tag="osb", bufs=5)
                    nc.scalar.copy(o_sb, pso[hl])
                    # 1/rowsum = exp(-ln(x)) on scalar, straight to bf16
                    lnr = pool.tile([65, 512], f32, name="lnr", tag="lnr", bufs=3)
                    nc.scalar.activation(lnr[64:65, :], o_sb[64:65, :], AF.Ln)
                    rsbf = pool.tile([65, 512], bf16, name="rsbf", tag="rsbf", bufs=5)
                    nc.scalar.activation(rsbf[64:65, :], lnr[64:65, :], AF.Exp,
                                         scale=-1.0)
                    tails.append((o_sb, rsbf))

                def finish():
                    for hl, (o_sb, rsbf) in enumerate(tails):
                        psb2 = psA_p.tile([64, 512], f32, name="psA", tag="psA")
                        nc.tensor.matmul(psb2, ones65[64:65, 0:64], rsbf[64:65, :],
                                         start=True, stop=True)
                        o_n = pool.tile([64, 512], f8, name="o_n", tag="on", bufs=3)
                        nc.vector.scalar_tensor_tensor(
                            out=o_n, in0=o_sb[0:64, :], scalar=SQ,
                            op0=ALU.mult, op1=ALU.mult, in1=psb2)
                        for half in range(2):
                            nc.sync.dma_start(
                                out=ap(a2a_in[b],
                                       (2 * n + half) * 128 * SUB + hl * 64 * SUB,
                                       [[SUB, 64], [1, SUB]]),
                                in_=o_n[:, half * SUB:(half + 1) * SUB])
                return finish

            def outproj_norm2(b):
                lo, hi = b * SUB, (b + 1) * SUB
                for m in range(8):
                    ps = psA_p.tile([128, SUB], f32, name="psA", tag="psA")
                    for a in range(4):
                        nc.tensor.matmul(
                            ps, wo_sb[:, 2 * a:2 * a + 2, m * 128:(m + 1) * 128],
                            a2a_sb[:, 2 * a:2 * a + 2, lo:hi],
                            start=(a == 0), stop=(a == 3), perf_mode=DR)
                    nc.vector.scalar_tensor_tensor(
                        out=x_fm[:, m, lo:hi], in0=ps, scalar=SO, op0=ALU.mult,
                        op1=ALU.add, in1=x_fm[:, m, lo:hi])
                psn2 = psA_p.tile([1, SUB], f32, name="psA", tag="psA")
                for m in range(8):
                    sq2 = pool.tile([128, SUB], bf16, name="sq2", tag="sq2", bufs=3)
                    nc.vector.tensor_mul(sq2, x_fm[:, m, lo:hi], x_fm[:, m, lo:hi])
                    nc.tensor.matmul(psn2, ones_col, sq2, start=(m == 0), stop=(m == 7))
                r2 = rstd2[:, lo:hi]
                nc.scalar.activation(r2, psn2, AF.Sqrt, scale=1.0 / D,
                                     bias=eps_t[0:1, :])
                r2b = pool.tile([1, SUB], bf16, name="r2b", tag="r1b", bufs=2)
                nc.vector.tensor_copy(r2b, r2)
                psb3 = psA_p.tile([128, SUB], f32, name="psA", tag="psA")
                nc.tensor.matmul(psb3, ones65[0:1, :], r2b, start=True, stop=True)
                rb3 = pool.tile([128, SUB], f32, name="rb3", tag="rb", bufs=2)
                nc.vector.reciprocal(rb3, psb3)
                for m in range(8):
                    nc.vector.tensor_mul(x2n[:, m, lo:hi], x_fm[:, m, lo:hi], rb3)

            def fire_a2a(b):
                nc.gpsimd.collective_compute(
                    "AllToAll", ALU.bypass, replica_groups=RG,
                    ins=[a2a_in[b][:].opt()], outs=[a2a_out[b][:].opt()])
                nc.sync.dma_start(out=a2a_sb[:, :, b * SUB:(b + 1) * SUB],
                                  in_=a2a_out[b].rearrange("r p t -> p r t"))

            def load_gu(half, gl):
                gf0 = half * 16 + gl * 8
                wg_t = pool.tile([128, 8, 1024], f8, name="wg_t", tag="w8", bufs=4)
                for dk in range(8):
                    nc.sync.dma_start(
                        out=wg_t[:, dk, :],
                        in_=wgT[dk * 128:(dk + 1) * 128, gf0 * 128:gf0 * 128 + 1024])
                wu_t = pool.tile([128, 8, 1024], f8, name="wu_t", tag="w8", bufs=4)
                for dk in range(8):
                    nc.sync.dma_start(
                        out=wu_t[:, dk, :],
                        in_=wuT[dk * 128:(dk + 1) * 128, gf0 * 128:gf0 * 128 + 1024])
                return wg_t, wu_t

            def gu_block(wg_t, wu_t, a_sb, gl, c0, c1):
                # F-tiles gl*8..gl*8+8 of the half, token cols [c0, c1)
                for q2 in range(2):
                    sgs = []
                    for mm in range(4):
                        fi = q2 * 4 + mm
                        psg = psA_p.tile([128, c1 - c0], f32, name="psA", tag="psA")
                        for a in range(4):
                            nc.tensor.matmul(
                                psg, wg_t[:, 2 * a:2 * a + 2, fi * 128:(fi + 1) * 128],
                                x2n[:, 2 * a:2 * a + 2, c0:c1],
                                start=(a == 0), stop=(a == 3), perf_mode=DR)
                        sg = pool.tile([128, c1 - c0], bf16, name="sg", tag="sg", bufs=5)
                        nc.scalar.activation(sg, psg, AF.Silu, scale=SG)
                        sgs.append(sg)
                    for mm in range(4):
                        fi = q2 * 4 + mm
                        psu = psA_p.tile([128, c1 - c0], f32, name="psA", tag="psA")
                        for a in range(4):
                            nc.tensor.matmul(
                                psu, wu_t[:, 2 * a:2 * a + 2, fi * 128:(fi + 1) * 128],
                                x2n[:, 2 * a:2 * a + 2, c0:c1],
                                start=(a == 0), stop=(a == 3), perf_mode=DR)
                        nc.vector.scalar_tensor_tensor(
                            out=a_sb[:, gl * 8 + fi, c0:c1], in0=psu, scalar=SU,
                            op0=ALU.mult, op1=ALU.mult, in1=sgs[mm])

            # --- batch 0: qkv + attention (normalization tails pipelined) ---
            pend = None
            for n in range(4):
                qkv_chunk(0, 2 * n)
                qkv_chunk(0, 2 * n + 1)
                fin = attention_n(0, n)
                if pend:
                    pend()
                pend = fin
                # hoist batch-1's qkv chunks under batch-0's long (scalar-
                # bound) attention blocks; AG1b has landed well before the
                # tensor queue reaches them, and batch-1's attention loop
                # then runs an uninterrupted exp stream
                if n >= 2:
                    for r in range(4 * (n - 2), 4 * (n - 2) + 4):
                        qkv_chunk(1, r)
            pend()
            fire_a2a(0)
            # --- batch 1 attention; batch-0 out_proj/norm2 overlaps its middle ---
            pend = None
            for n in range(4):
                fin = attention_n(1, n)
                if pend:
                    pend()
                pend = fin
                if n == 1:
                    outproj_norm2(0)
            # prefetch first FFN gate/up weights (DMA only), finish the last
            # attention tail, fire the second AllToAll, then fill its window
            # with the first gate/up block on batch-0 tokens
            a_sb0 = pool.tile([128, 16, TLOC], bf16, name="a_sb", tag="asb", bufs=2)
            wg0, wu0 = load_gu(0, 0)
            pend()
            fire_a2a(1)
            gu_block(wg0, wu0, a_sb0, 0, 0, SUB)
            outproj_norm2(1)

            # ============ FFN (two F-halves; down accumulated into x_fm) ============
            for half in range(2):
                a_sb = a_sb0 if half == 0 else pool.tile(
                    [128, 16, TLOC], bf16, name="a_sb", tag="asb", bufs=2)
                for gl in range(2):
                    if half == 0 and gl == 0:
                        gu_block(wg0, wu0, a_sb, 0, SUB, TLOC)
                        continue
                    wg_t, wu_t = load_gu(half, gl)
                    gu_block(wg_t, wu_t, a_sb, gl, 0, TLOC)
                wds = []
                for dh in range(2):
                    wd_t = pool.tile([128, 8, 1024], f8, name="wd_t", tag="w8", bufs=4)
                    for f8_ in range(8):
                        fk = half * 16 + dh * 8 + f8_
                        nc.sync.dma_start(out=wd_t[:, f8_, :],
                                          in_=wdT[fk * 128:(fk + 1) * 128, :])
                    wds.append(wd_t)
                for mo in range(8):
                    psd = psA_p.tile([128, 512], f32, name="psA", tag="psA")
                    for ff in range(16):
                        nc.tensor.matmul(
                            psd, wds[ff // 8][:, ff % 8, mo * 128:(mo + 1) * 128],
                            a_sb[:, ff, :], start=(ff == 0), stop=(ff == 15))
                    nc.vector.scalar_tensor_tensor(
                        out=x_fm[:, mo, :], in0=psd, scalar=SD, op0=ALU.mult,
                        op1=ALU.add, in1=x_fm[:, mo, :])
                    # once a 4-tile do-group is final (second F-half), emit its
                    # output transposes + stores immediately
                    if half == 1 and mo % 4 == 3:
                        doh = mo // 4
                        for tt in range(4):
                            ps = psA_p.tile([128, 512], f32, name="psA", tag="psA")
                            for mm in range(4):
                                nc.tensor.transpose(
                                    ps[:, mm * 128:(mm + 1) * 128],
                                    x_fm[:, doh * 4 + mm, tt * 128:(tt + 1) * 128],
                                    id_f32)
                            ob = pool.tile([128, 512], f32, name="ob", tag="f2k",
                                           bufs=3)
                            nc.vector.tensor_copy(ob, ps)
                            nc.sync.dma_start(
                                out=out_d[tt * 128:(tt + 1) * 128,
                                          doh * 512:(doh + 1) * 512],
                                in_=ob)
    _legalize_multiwaits(nc)
    return nc


def _quant(w, s):
    # host-side ternary quantization (exact in fp8-e4m3); matches
    # jnp.clip(jnp.round(W/s), -1, 1) including round-half-to-even
    return np.clip(np.round(w / s), -1.0, 1.0).astype(np.float32)


def _prepare(inputs):
    import ml_dtypes
    f8 = ml_dtypes.float8_e4m3
    x = np.asarray(inputs["x"], np.float32).reshape(BT, D)
    qkv_w = np.asarray(inputs["qkv_w"], np.float32)
    out_w = np.asarray(inputs["out_w"], np.float32)
    gate_w = np.asarray(inputs["gate_w"], np.float32)
    up_w = np.asarray(inputs["up_w"], np.float32)
    down_w = np.asarray(inputs["down_w"], np.float32)
    ln1 = np.asarray(inputs["ln1_w"], np.float32)
    ln2 = np.asarray(inputs["ln2_w"], np.float32)

    scales = {
        "qkv": max(np.mean(np.abs(qkv_w), dtype=np.float32), np.float32(1e-5)),
        "out": max(np.mean(np.abs(out_w), dtype=np.float32), np.float32(1e-5)),
        "gate": max(np.mean(np.abs(gate_w), dtype=np.float32), np.float32(1e-5)),
        "up": max(np.mean(np.abs(up_w), dtype=np.float32), np.float32(1e-5)),
        "down": max(np.mean(np.abs(down_w), dtype=np.float32), np.float32(1e-5)),
    }
    qkv_q = _quant(qkv_w, scales["qkv"]) * ln1[None, :]     # fold g1 into cols (d)
    out_q = _quant(out_w, scales["out"])
    gate_q = _quant(gate_w, scales["gate"]) * ln2[None, :]  # fold g2 into cols (d)
    up_q = _quant(up_w, scales["up"]) * ln2[None, :]
    down_q = _quant(down_w, scales["down"])

    # shared (identical on all cores), matmul-ready transposed layouts
    # woT[p, r, do] = out_q[do, r*128+p]
    woT = np.ascontiguousarray(
        out_q.T.reshape(8, 128, 1024).transpose(1, 0, 2).reshape(128, 8192)
    ).astype(f8)
    wgT = np.ascontiguousarray(gate_q.T).astype(f8)         # [1024 d, 4096 F]
    wuT = np.ascontiguousarray(up_q.T).astype(f8)
    wdT = np.ascontiguousarray(down_q.T).astype(f8)         # [4096 F, 1024 do]

    in_maps = []
    for c in range(NC_):
        # wqkvT[p, m, dk, f] = qkv_q[m*1024 + c*128 + f, dk*128 + p]
        wq = np.stack([
            qkv_q[m * 1024 + c * 128: m * 1024 + (c + 1) * 128, :]
            .T.reshape(8, 128, 128).transpose(1, 0, 2)
            for m in range(3)], axis=1)                      # [128, 3, 8, 128]
        in_maps.append({
            "x_slice": np.ascontiguousarray(np.concatenate(
                [x[SUB * c: SUB * (c + 1)],
                 x[T + SUB * c: T + SUB * (c + 1)]], axis=0)),
            "wqkvT": np.ascontiguousarray(wq.reshape(128, 3072)).astype(f8),
            "woT": woT,
            "wgT": wgT,
            "wuT": wuT,
            "wdT": wdT,
        })
    return scales, in_maps


def run(inputs, trace=False):
    from concourse.bass_utils import run_bass_kernel_spmd
    scales, in_maps = _prepare(inputs)
    nc = _build(scales)
    res = run_bass_kernel_spmd(nc, in_maps, list(range(NC_)), trace=trace)
    out = np.empty((BT, D), np.float32)
    for c in range(NC_):
        o = np.asarray(res.results[c]["out"])
        out[SUB * c: SUB * (c + 1)] = o[0:SUB]
        out[T + SUB * c: T + SUB * (c + 1)] = o[SUB:2 * SUB]
    return out.reshape(B, T, D), res


def kernel(**inputs):
    out, _ = run(inputs, trace=False)
    return out
